# revision 7
# baseline (speedup 1.0000x reference)
"""Trainium2 Bass kernel for nn_AbstractFullyConnected (DeepPoly-style abstract MLP).

Network: 784 -> 4096 -> 4096 -> 4096 -> 10, batch=1, with box-bound propagation.

Math (exact interval arithmetic): with c=(low+high)/2, r=(high-low)/2:
    W_pos@low + W_neg@high = W@c - |W|@r
    W_pos@high + W_neg@low = W@c + |W|@r
After each AbstractRelu boundary low=0, so c' = r' = high'/2.

Precision: weights and stationary vectors are split into bf16 hi+lo pairs
(W = Whi + Wlo).  Three PE passes per layer, with the small vectors as the
3/6-column *stationary* operand and weight tiles streamed as the *moving*
operand at 1 column/cycle:
    pass A: Whi @ [xhi,chi,chi | xlo,clo,clo] -> psum rows 0-2 (hi) + 3-5 (lo)
    pass B: Wlo @ [xhi,chi,chi]               -> psum rows 0-2
    pass C: |Whi| @ [0,-rhi,rhi | 0,-rlo,rlo] -> rows 1,2 (-/+v) + 4,5
(|W|'s bf16-lo correction is dropped: bf16 rounding residuals are mean-zero
and r>0, so the error cancels to ~1e-5 relative overall.)  |Whi| is computed
on-chip so HBM traffic stays at 4 bytes/weight.  Bias enters as two extra
contraction rows (bhi, blo) against an all-ones stationary.

Sharding (tensor parallel over 8 cores, ONE collective total):
    L1 row-sharded (512 rows/core) -> local elementwise ReLU boundary ->
    L2 column-sharded (512 inputs/core, bias pre-divided by 8) ->
    AllReduce of the (6, 4096) partials -> replicated full-vector boundary ->
    L3 row-sharded -> per-core output shards.
The final boundary and the tiny 10x4096 layer 4 run on the host in numpy.
"""

import numpy as np

N_CORES = 8
MEAN = np.float32(0.1307)
STD = np.float32(0.3081)
EPS = np.float32(1e-6)
MS = 512            # rows per core for row-sharded layers
K1, K1P, T1 = 784, 896, 7
K23 = 4096
T2 = 4              # local contraction chunks for col-sharded L2 (512/128)
T3 = 32             # contraction chunks for L3 (4096/128)
NB = 8              # psum banks / N-slices for L2 output (4096/512)

_CACHE = {}


def _build_nc():
    import concourse.bacc as bacc
    import concourse.mybir as mybir
    import concourse.tile as tile

    F32 = mybir.dt.float32
    BF16 = mybir.dt.bfloat16
    ALU = mybir.AluOpType
    ACTF = mybir.ActivationFunctionType

    nc = bacc.Bacc("TRN2", target_bir_lowering=False, debug=False,
                   num_devices=N_CORES)

    stat1_d = nc.dram_tensor("stat1", [128, 12 * T1], BF16, kind="ExternalInput")
    brow_d = nc.dram_tensor("brow", [2, MS + K23 + 3], BF16, kind="ExternalInput")
    wh1_d = nc.dram_tensor("wh1", [128, T1 * MS], BF16, kind="ExternalInput")
    wl1_d = nc.dram_tensor("wl1", [128, T1 * MS], BF16, kind="ExternalInput")
    wh2_d = nc.dram_tensor("wh2", [128, T2 * K23], BF16, kind="ExternalInput")
    wl2_d = nc.dram_tensor("wl2", [128, T2 * K23], BF16, kind="ExternalInput")
    wh3_d = nc.dram_tensor("wh3", [128, T3 * MS], BF16, kind="ExternalInput")
    wl3_d = nc.dram_tensor("wl3", [128, T3 * MS], BF16, kind="ExternalInput")
    out_d = nc.dram_tensor("out", [6, MS], F32, kind="ExternalOutput")

    with tile.TileContext(nc) as tc:
        with (
            tc.tile_pool(name="wp", bufs=1) as wp,
            tc.tile_pool(name="sp", bufs=1) as sp,
            tc.tile_pool(name="absp", bufs=8) as absp,
            tc.tile_pool(name="pp", bufs=1, space="PSUM") as pp,
            tc.tile_pool(name="dp", bufs=1, space="DRAM") as dp,
        ):
            # ---- input DMAs (HWDGE/SP ring; trace order == drain order) ----
            stat1 = sp.tile([128, 12 * T1], BF16, tag="stat1")
            brow = sp.tile([2, MS + K23 + 3], BF16, tag="brow")
            nc.sync.dma_start(stat1[:], stat1_d[:])
            nc.sync.dma_start(brow[:], brow_d[:])
            wh1 = wp.tile([128, T1 * MS], BF16, tag="wh1")
            wl1 = wp.tile([128, T1 * MS], BF16, tag="wl1")
            nc.sync.dma_start(wh1[:], wh1_d[:])
            nc.sync.dma_start(wl1[:], wl1_d[:])

            CH2 = T2 * K23 // 2
            CH3 = T3 * MS // 2
            wh2 = [wp.tile([128, CH2], BF16, name=f"wh2_{i}", tag=f"wh2_{i}")
                   for i in range(2)]
            wl2 = [wp.tile([128, CH2], BF16, name=f"wl2_{i}", tag=f"wl2_{i}")
                   for i in range(2)]
            wh3 = [wp.tile([128, CH3], BF16, name=f"wh3_{i}", tag=f"wh3_{i}")
                   for i in range(2)]
            wl3 = [wp.tile([128, CH3], BF16, name=f"wl3_{i}", tag=f"wl3_{i}")
                   for i in range(2)]
            for i in range(2):
                nc.sync.dma_start(wh2[i][:], wh2_d[:, i * CH2:(i + 1) * CH2])
            for i in range(2):
                nc.sync.dma_start(wl2[i][:], wl2_d[:, i * CH2:(i + 1) * CH2])
            for i in range(2):
                nc.sync.dma_start(wh3[i][:], wh3_d[:, i * CH3:(i + 1) * CH3])
            for i in range(2):
                nc.sync.dma_start(wl3[i][:], wl3_d[:, i * CH3:(i + 1) * CH3])

            def slice_of(tiles, ch, off):
                return tiles[off // ch][:, off % ch:off % ch + MS]

            n_abs = [0]

            def abs_chunk(src_ap):
                t_ = absp.tile([128, MS], BF16, name=f"abs{n_abs[0]}", tag="abs")
                if n_abs[0] % 2 == 0:
                    nc.vector.scalar_tensor_tensor(t_[:], src_ap, -1.0, src_ap,
                                                   ALU.mult, ALU.max)
                else:
                    nc.scalar.activation(t_[:], src_ap, ACTF.Abs)
                n_abs[0] += 1
                return t_

            ones = brow[0:2, MS + K23:MS + K23 + 3]

            def layer_row_sharded(wstat, rstat, wht, wlt, wch, ntau, ps_tag,
                                  bias_off):
                """Row-sharded layer -> psum [6, MS]."""
                ps = pp.tile([6, MS], F32, name=f"ps_{ps_tag}", tag=ps_tag)
                for t_ in range(ntau):
                    nc.tensor.matmul(ps[:], wstat[:, 6 * t_:6 * t_ + 6],
                                     slice_of(wht, wch, t_ * MS),
                                     start=(t_ == 0), stop=False)
                for t_ in range(ntau):
                    nc.tensor.matmul(ps[0:3, :], wstat[:, 6 * t_:6 * t_ + 3],
                                     slice_of(wlt, wch, t_ * MS),
                                     start=False, stop=False)
                if bias_off is not None:
                    nc.tensor.matmul(ps[0:3, :], ones,
                                     brow[0:2, bias_off:bias_off + MS],
                                     start=False, stop=False)
                for t_ in range(ntau):
                    a = abs_chunk(slice_of(wht, wch, t_ * MS))
                    nc.tensor.matmul(ps[:], rstat[:, 6 * t_:6 * t_ + 6], a[:],
                                     start=False, stop=(t_ == ntau - 1))
                return ps

            def boundary(vec, Tl, wstat, rstat, pref):
                """vec [128, 6*Tl] f32 col-blocks (z|lo|hi, hi-part then
                lo-part) -> next-layer bf16 stationaries."""
                def tmp(n):
                    return sp.tile([128, Tl], F32, name=f"{pref}{n}",
                                   tag=f"{pref}{n}")
                X, L_, H_ = tmp("X"), tmp("L"), tmp("H")
                nc.vector.tensor_add(X[:], vec[:, 0:Tl], vec[:, 3 * Tl:4 * Tl])
                nc.vector.tensor_add(L_[:], vec[:, Tl:2 * Tl],
                                     vec[:, 4 * Tl:5 * Tl])
                nc.vector.tensor_add(H_[:], vec[:, 2 * Tl:3 * Tl],
                                     vec[:, 5 * Tl:6 * Tl])
                d0, d1, r0, r1 = tmp("d0"), tmp("d1"), tmp("r0"), tmp("r1")
                u1, u2, s_, xr = tmp("u1"), tmp("u2"), tmp("s"), tmp("xr")
                cf, hf, cl = tmp("cf"), tmp("hf"), tmp("cl")
                nc.vector.tensor_sub(d0[:], H_[:], L_[:])
                nc.vector.tensor_scalar_add(d1[:], d0[:], float(EPS))
                nc.vector.reciprocal(r1[:], d1[:])
                nc.vector.reciprocal(r0[:], d0[:])
                nc.vector.tensor_mul(u1[:], H_[:], r1[:])
                nc.vector.tensor_mul(u2[:], L_[:], r0[:])
                nc.vector.tensor_add(s_[:], u1[:], u2[:])
                stt = nc.vector.scalar_tensor_tensor
                # c' = r' = (H*0.5)*s ; x' = relu(X)
                stt(cf[:], H_[:], 0.5, s_[:], ALU.mult, ALU.mult)
                nc.vector.tensor_relu(xr[:], X[:])
                n6 = 6 * Tl
                # wstat cols per tau: (xhi, chi, chi, xlo, clo, clo)
                nc.vector.tensor_copy(wstat[:, 0:n6:6], xr[:])        # xhi
                nc.vector.tensor_copy(hf[:], wstat[:, 0:n6:6])        # back to f32
                nc.vector.tensor_sub(wstat[:, 3:n6:6], xr[:], hf[:])  # xlo
                nc.vector.tensor_copy(wstat[:, 1:n6:6], cf[:])        # chi
                nc.vector.tensor_copy(hf[:], wstat[:, 1:n6:6])
                nc.vector.tensor_copy(wstat[:, 2:n6:6], hf[:])
                nc.vector.tensor_sub(cl[:], cf[:], hf[:])             # clo f32
                nc.vector.tensor_copy(wstat[:, 4:n6:6], cl[:])
                nc.vector.tensor_copy(wstat[:, 5:n6:6], cl[:])
                # rstat cols per tau: (0, -rhi, rhi, 0, -rlo, rlo); r == c
                nc.vector.tensor_scalar_mul(rstat[:, 0:n6:6], cf[:], 0.0)
                nc.vector.tensor_scalar_mul(rstat[:, 3:n6:6], cf[:], 0.0)
                nc.vector.tensor_scalar_mul(rstat[:, 1:n6:6], hf[:], -1.0)
                nc.vector.tensor_copy(rstat[:, 2:n6:6], hf[:])
                nc.vector.tensor_scalar_mul(rstat[:, 4:n6:6], cl[:], -1.0)
                nc.vector.tensor_copy(rstat[:, 5:n6:6], cl[:])

            # ================= layer 1 (row-sharded) =================
            ps1 = layer_row_sharded(stat1[:, 0:6 * T1], stat1[:, 6 * T1:12 * T1],
                                    [wh1], [wl1], T1 * MS, T1, "bank2_0", 0)
            sb1 = sp.tile([6, MS], F32, tag="sb1")
            nc.scalar.activation(sb1[:], ps1[:], ACTF.Copy)
            # local shard -> [128, 6*T2] layout (partition-major reshape DMAs)
            vec1 = sp.tile([128, 6 * T2], F32, tag="vec1")
            for v in range(6):
                nc.gpsimd.dma_start(vec1[:, v * T2:(v + 1) * T2], sb1[v:v + 1, :])
            wstat2 = sp.tile([128, 6 * T2], BF16, tag="wstat2")
            rstat2 = sp.tile([128, 6 * T2], BF16, tag="rstat2")
            boundary(vec1, T2, wstat2, rstat2, "b1")

            # ================= layer 2 (column-sharded) =================
            ps2 = [pp.tile([6, MS], F32, name=f"ps2_{n}", tag=f"bank2_{n}")
                   for n in range(NB)]
            for t_ in range(T2):
                for n in range(NB):
                    nc.tensor.matmul(ps2[n][:], wstat2[:, 6 * t_:6 * t_ + 6],
                                     slice_of(wh2, CH2, t_ * K23 + n * MS),
                                     start=(t_ == 0), stop=False)
            for t_ in range(T2):
                for n in range(NB):
                    nc.tensor.matmul(ps2[n][0:3, :], wstat2[:, 6 * t_:6 * t_ + 3],
                                     slice_of(wl2, CH2, t_ * K23 + n * MS),
                                     start=False, stop=False)
            for n in range(NB):
                nc.tensor.matmul(ps2[n][0:3, :], ones,
                                 brow[0:2, MS + n * MS:MS + (n + 1) * MS],
                                 start=False, stop=False)
            for t_ in range(T2):
                for n in range(NB):
                    a = abs_chunk(slice_of(wh2, CH2, t_ * K23 + n * MS))
                    nc.tensor.matmul(ps2[n][:], rstat2[:, 6 * t_:6 * t_ + 6],
                                     a[:], start=False, stop=(t_ == T2 - 1))
            sb2 = sp.tile([6, K23], F32, tag="sb2")
            for n in range(NB):
                nc.vector.tensor_copy(sb2[:, n * MS:(n + 1) * MS], ps2[n][:])
            # ---- the one collective: AllReduce of (6, 4096) partials ----
            ar_in = dp.tile([6, K23], F32, tag="ar_in")
            ar_out = dp.tile([6, K23], F32, tag="ar_out")
            nc.scalar.dma_start(ar_in[:], sb2[:])
            nc.gpsimd.collective_compute(
                "AllReduce", mybir.AluOpType.add,
                replica_groups=[list(range(N_CORES))],
                ins=[ar_in[:]], outs=[ar_out[:]],
            )
            vec2 = sp.tile([128, 6 * T3], F32, tag="vec2")
            nc.scalar.dma_start(
                vec2[:], ar_out.rearrange("v (p t) -> p v t", p=128, t=T3))
            wstat3 = sp.tile([128, 6 * T3], BF16, tag="wstat3")
            rstat3 = sp.tile([128, 6 * T3], BF16, tag="rstat3")
            boundary(vec2, T3, wstat3, rstat3, "b2")

            # ================= layer 3 (row-sharded) =================
            ps3 = layer_row_sharded(wstat3, rstat3, wh3, wl3, CH3, T3,
                                    "bank2_1", None)
            sb3 = sp.tile([6, MS], F32, tag="sb3")
            nc.scalar.activation(sb3[:], ps3[:], ACTF.Copy)
            nc.scalar.dma_start(out_d[:], sb3[:])
    nc.compile()
    return nc


def _bf_split(a):
    import ml_dtypes
    hi = a.astype(ml_dtypes.bfloat16)
    lo = (a - hi.astype(np.float32)).astype(ml_dtypes.bfloat16)
    return hi, lo


def _row_prep(shard, kp):
    """(rows, K) slice of W -> [128, T*rows] with wt[p, tau*rows+m] =
    W.T[p*T+tau, m], zero-padded to kp contraction rows; bf16 hi/lo pair."""
    rows, k = shard.shape
    arr = np.zeros((kp, rows), np.float32)
    arr[:k] = shard.T
    t = kp // 128
    arr = np.ascontiguousarray(arr.reshape(128, t * rows))
    return _bf_split(arr)


def _prep_inputs(x, low, high, Ws, bs):
    import ml_dtypes
    BF = ml_dtypes.bfloat16

    xn = (x - MEAN) / STD
    ln = (low - MEAN) / STD
    hn = (high - MEAN) / STD
    c0 = (ln + hn) * np.float32(0.5)
    r0 = (hn - ln) * np.float32(0.5)

    def padv(v):
        p = np.zeros(K1P, np.float32)
        p[:K1] = v
        return p.reshape(128, T1)

    xs, cs, rs = padv(xn), padv(c0), padv(r0)
    stat1 = np.zeros((128, 12 * T1), BF)
    n6 = 6 * T1
    xhi, xlo = _bf_split(xs)
    chi, clo = _bf_split(cs)
    rhi, rlo = _bf_split(rs)
    stat1[:, 0:n6:6] = xhi
    stat1[:, 1:n6:6] = chi
    stat1[:, 2:n6:6] = chi
    stat1[:, 3:n6:6] = xlo
    stat1[:, 4:n6:6] = clo
    stat1[:, 5:n6:6] = clo
    stat1[:, n6 + 1::6] = -(rhi.astype(np.float32))
    stat1[:, n6 + 2::6] = rhi
    stat1[:, n6 + 4::6] = -(rlo.astype(np.float32))
    stat1[:, n6 + 5::6] = rlo

    b2_8 = bs[1] / np.float32(8.0)

    in_maps = []
    for c in range(N_CORES):
        sl = slice(c * MS, (c + 1) * MS)
        brow = np.zeros((2, MS + K23 + 3), BF)
        b1hi, b1lo = _bf_split(bs[0][sl])
        b2hi, b2lo = _bf_split(b2_8)
        brow[0, 0:MS] = b1hi
        brow[1, 0:MS] = b1lo
        brow[0, MS:MS + K23] = b2hi
        brow[1, MS:MS + K23] = b2lo
        brow[0, MS + K23:] = np.ones(3, BF)
        brow[1, MS + K23:] = np.ones(3, BF)

        wh1, wl1 = _row_prep(Ws[0][sl], K1P)
        # L2 column shard: wt[p, tau*4096 + m] = W2[m, c*512 + p*4 + tau]
        arr2 = np.ascontiguousarray(Ws[1][:, sl].T)        # (512, 4096)
        arr2 = np.ascontiguousarray(arr2.reshape(128, T2 * K23))
        wh2, wl2 = _bf_split(arr2)
        wh3, wl3 = _row_prep(Ws[2][sl], K23)
        in_maps.append({
            "stat1": stat1, "brow": brow,
            "wh1": wh1, "wl1": wl1,
            "wh2": wh2, "wl2": wl2,
            "wh3": wh3, "wl3": wl3,
        })
    return in_maps


def _run_device(in_maps, trace=False):
    from concourse.bass_utils import run_bass_kernel_spmd

    if "nc" not in _CACHE:
        _CACHE["nc"] = _build_nc()
    return run_bass_kernel_spmd(_CACHE["nc"], in_maps,
                                core_ids=list(range(N_CORES)), trace=trace)


def kernel(x, low, high, Ws, bs, _trace=False, _res_out=None):
    x = np.asarray(x, np.float32).reshape(-1)
    low = np.asarray(low, np.float32).reshape(-1)
    high = np.asarray(high, np.float32).reshape(-1)
    Ws = [np.ascontiguousarray(np.asarray(W, np.float32)) for W in Ws]
    bs = [np.asarray(b, np.float32) for b in bs]

    in_maps = _prep_inputs(x, low, high, Ws, bs)
    res = _run_device(in_maps, trace=_trace)
    if _res_out is not None:
        _res_out.append(res)

    outs = [res.results[c]["out"] for c in range(N_CORES)]
    z3 = np.concatenate([o[0] + o[3] for o in outs])
    lo3 = np.concatenate([o[1] + o[4] for o in outs])
    hi3 = np.concatenate([o[2] + o[5] for o in outs])

    # layer-3 bias + ReLU boundary (host, fp32)
    b3 = bs[2]
    x3 = z3 + b3
    lo = lo3 + b3
    hi = hi3 + b3
    d0 = hi - lo
    slope = hi / (d0 + EPS)
    ubint = lo * hi / d0
    hi2 = slope * hi + ubint
    x3p = np.maximum(x3, np.float32(0))
    c3 = hi2 * np.float32(0.5)

    # layer 4 (10 x 4096) on host
    W4, b4 = Ws[3], bs[3]
    z4 = W4 @ x3p
    u4 = W4 @ c3
    v4 = np.abs(W4) @ c3
    x_out = (z4 + b4).reshape(1, -1).astype(np.float32)
    low_out = (u4 - v4 + b4).astype(np.float32)
    high_out = (u4 + v4 + b4).astype(np.float32)
    return (x_out, low_out, high_out)


# revision 10
# speedup vs baseline: 2.0960x; 2.0960x over previous
"""Trainium2 Bass kernel for nn_AbstractFullyConnected (DeepPoly-style abstract MLP).

Network: 784 -> 4096 -> 4096 -> 4096 -> 10, batch=1, with box-bound propagation.

Math (exact interval arithmetic): with c=(low+high)/2, r=(high-low)/2:
    W_pos@low + W_neg@high = W@c - |W|@r
    W_pos@high + W_neg@low = W@c + |W|@r
After each AbstractRelu boundary low=0, so c' = r' = high'/2.

Precision: weights and stationary vectors are split into bf16 hi+lo pairs
(W = Whi + Wlo).  Three PE passes per layer, with the small vectors as the
3/6-column *stationary* operand and weight tiles streamed as the *moving*
operand at 1 column/cycle:
    pass A: Whi @ [xhi,chi,chi | xlo,clo,clo] -> psum rows 0-2 (hi) + 3-5 (lo)
    pass B: Wlo @ [xhi,chi,chi]               -> psum rows 0-2
    pass C: |Whi| @ [0,-rhi,rhi | 0,-rlo,rlo] -> rows 1,2 (-/+v) + 4,5
(|W|'s bf16-lo correction is dropped: bf16 rounding residuals are mean-zero
and r>0, so the error cancels to ~1e-5 relative overall.)  |Whi| is computed
on-chip so HBM traffic stays at 4 bytes/weight.  Bias enters as two extra
contraction rows (bhi, blo) against an all-ones stationary.

Sharding (tensor parallel over 8 cores, ONE collective total):
    L1 row-sharded (512 rows/core) -> local elementwise ReLU boundary ->
    L2 column-sharded (512 inputs/core, bias pre-divided by 8) ->
    AllReduce of the (6, 4096) partials -> replicated full-vector boundary ->
    L3 row-sharded -> per-core output shards.
The final boundary and the tiny 10x4096 layer 4 run on the host in numpy.
"""

import numpy as np

N_CORES = 8
MEAN = np.float32(0.1307)
STD = np.float32(0.3081)
EPS = np.float32(1e-6)
MS = 512            # rows per core for row-sharded layers
K1, K1P, T1 = 784, 896, 7
K23 = 4096
T2 = 4              # local contraction chunks for col-sharded L2 (512/128)
T3 = 32             # contraction chunks for L3 (4096/128)
NB = 8              # psum banks / N-slices for L2 output (4096/512)

_CACHE = {}


def _build_nc():
    import concourse.bacc as bacc
    import concourse.mybir as mybir
    import concourse.tile as tile

    F32 = mybir.dt.float32
    BF16 = mybir.dt.bfloat16
    ALU = mybir.AluOpType
    ACTF = mybir.ActivationFunctionType

    nc = bacc.Bacc("TRN2", target_bir_lowering=False, debug=False,
                   num_devices=N_CORES)

    stat1_d = nc.dram_tensor("stat1", [128, 12 * T1], BF16, kind="ExternalInput")
    brow_d = nc.dram_tensor("brow", [2, MS + K23 + 3], BF16, kind="ExternalInput")
    wh1_d = nc.dram_tensor("wh1", [128, T1 * MS], BF16, kind="ExternalInput")
    wl1_d = nc.dram_tensor("wl1", [128, T1 * MS], BF16, kind="ExternalInput")
    wh2_d = nc.dram_tensor("wh2", [128, T2 * K23], BF16, kind="ExternalInput")
    wl2_d = nc.dram_tensor("wl2", [128, T2 * K23], BF16, kind="ExternalInput")
    wh3_d = nc.dram_tensor("wh3", [128, T3 * MS], BF16, kind="ExternalInput")
    wl3_d = nc.dram_tensor("wl3", [128, T3 * MS], BF16, kind="ExternalInput")
    out_d = nc.dram_tensor("out", [6, MS], F32, kind="ExternalOutput")

    with tile.TileContext(nc) as tc:
        with (
            tc.tile_pool(name="wp", bufs=1) as wp,
            tc.tile_pool(name="sp", bufs=1) as sp,
            tc.tile_pool(name="absp", bufs=6) as absp,
            tc.tile_pool(name="pp", bufs=1, space="PSUM") as pp,
            tc.tile_pool(name="dp", bufs=1, space="DRAM") as dp,
        ):
            # ---- input DMAs (HWDGE/SP ring; trace order == drain order) ----
            stat1 = sp.tile([128, 12 * T1], BF16, tag="stat1")
            brow = sp.tile([2, MS + K23 + 3], BF16, tag="brow")
            nc.sync.dma_start(stat1[:], stat1_d[:])
            nc.sync.dma_start(brow[:], brow_d[:])
            wh1 = wp.tile([128, T1 * MS], BF16, tag="wh1")
            wl1 = wp.tile([128, T1 * MS], BF16, tag="wl1")
            nc.sync.dma_start(wh1[:], wh1_d[:])
            nc.sync.dma_start(wl1[:], wl1_d[:])

            CH2 = T2 * K23 // 2
            CH3 = T3 * MS // 2
            wh2 = [wp.tile([128, CH2], BF16, name=f"wh2_{i}", tag=f"wh2_{i}")
                   for i in range(2)]
            wl2 = [wp.tile([128, CH2], BF16, name=f"wl2_{i}", tag=f"wl2_{i}")
                   for i in range(2)]
            wh3 = [wp.tile([128, CH3], BF16, name=f"wh3_{i}", tag=f"wh3_{i}")
                   for i in range(2)]
            wl3 = [wp.tile([128, CH3], BF16, name=f"wl3_{i}", tag=f"wl3_{i}")
                   for i in range(2)]
            for i in range(2):
                nc.sync.dma_start(wh2[i][:], wh2_d[:, i * CH2:(i + 1) * CH2])
            for i in range(2):
                nc.sync.dma_start(wl2[i][:], wl2_d[:, i * CH2:(i + 1) * CH2])
            for i in range(2):
                nc.sync.dma_start(wh3[i][:], wh3_d[:, i * CH3:(i + 1) * CH3])
            for i in range(2):
                nc.sync.dma_start(wl3[i][:], wl3_d[:, i * CH3:(i + 1) * CH3])

            def slice_of(tiles, ch, off):
                return tiles[off // ch][:, off % ch:off % ch + MS]

            ah3 = [wp.tile([128, CH3], BF16, name=f"ah3_{i}", tag=f"ah3_{i}")
                   for i in range(2)]

            n_abs = [0]

            def abs_chunk(src_ap):
                t_ = absp.tile([128, MS], BF16, name=f"abs{n_abs[0]}", tag="abs")
                if n_abs[0] % 2 == 0:
                    nc.vector.scalar_tensor_tensor(t_[:], src_ap, -1.0, src_ap,
                                                   ALU.mult, ALU.max)
                else:
                    nc.scalar.activation(t_[:], src_ap, ACTF.Abs)
                n_abs[0] += 1
                return t_

            ones = brow[0:2, MS + K23:MS + K23 + 3]

            def layer_row_sharded(wstat, rstat, wht, wlt, wch, ntau, ps_tag,
                                  bias_off, pre_abs=None):
                """Row-sharded layer -> psum [6, MS]."""
                ps = pp.tile([6, MS], F32, name=f"ps_{ps_tag}", tag=ps_tag)
                for t_ in range(ntau):
                    nc.tensor.matmul(ps[:], wstat[:, 6 * t_:6 * t_ + 6],
                                     slice_of(wht, wch, t_ * MS),
                                     start=(t_ == 0), stop=False)
                for t_ in range(ntau):
                    nc.tensor.matmul(ps[0:3, :], wstat[:, 6 * t_:6 * t_ + 3],
                                     slice_of(wlt, wch, t_ * MS),
                                     start=False, stop=False)
                if bias_off is not None:
                    nc.tensor.matmul(ps[0:3, :], ones,
                                     brow[0:2, bias_off:bias_off + MS],
                                     start=False, stop=False)
                for t_ in range(ntau):
                    if pre_abs is not None:
                        a = slice_of(pre_abs, wch, t_ * MS)
                    else:
                        a = abs_chunk(slice_of(wht, wch, t_ * MS))[:]
                    nc.tensor.matmul(ps[:], rstat[:, 6 * t_:6 * t_ + 6], a,
                                     start=False, stop=(t_ == ntau - 1))
                return ps

            def boundary(vec, Tl, wstat, rstat, pref):
                """vec [128, 6*Tl] f32 col-blocks (z|lo|hi, hi-part then
                lo-part) -> next-layer bf16 stationaries."""
                def tmp(n):
                    return sp.tile([128, Tl], F32, name=f"{pref}{n}",
                                   tag=f"{pref}{n}")
                X, L_, H_ = tmp("X"), tmp("L"), tmp("H")
                nc.vector.tensor_add(X[:], vec[:, 0:Tl], vec[:, 3 * Tl:4 * Tl])
                nc.vector.tensor_add(L_[:], vec[:, Tl:2 * Tl],
                                     vec[:, 4 * Tl:5 * Tl])
                nc.vector.tensor_add(H_[:], vec[:, 2 * Tl:3 * Tl],
                                     vec[:, 5 * Tl:6 * Tl])
                d0, d1, r0, r1 = tmp("d0"), tmp("d1"), tmp("r0"), tmp("r1")
                u1, u2, s_, xr = tmp("u1"), tmp("u2"), tmp("s"), tmp("xr")
                cf, hf, cl = tmp("cf"), tmp("hf"), tmp("cl")
                nc.vector.tensor_sub(d0[:], H_[:], L_[:])
                nc.vector.tensor_scalar_add(d1[:], d0[:], float(EPS))
                nc.vector.reciprocal(r1[:], d1[:])
                nc.vector.reciprocal(r0[:], d0[:])
                nc.vector.tensor_mul(u1[:], H_[:], r1[:])
                nc.vector.tensor_mul(u2[:], L_[:], r0[:])
                nc.vector.tensor_add(s_[:], u1[:], u2[:])
                stt = nc.vector.scalar_tensor_tensor
                # c' = r' = (H*0.5)*s ; x' = relu(X)
                stt(cf[:], H_[:], 0.5, s_[:], ALU.mult, ALU.mult)
                nc.vector.tensor_relu(xr[:], X[:])
                n6 = 6 * Tl
                # wstat cols per tau: (xhi, chi, chi, xlo, clo, clo)
                nc.vector.tensor_copy(wstat[:, 0:n6:6], xr[:])        # xhi
                nc.vector.tensor_copy(hf[:], wstat[:, 0:n6:6])        # back to f32
                nc.vector.tensor_sub(wstat[:, 3:n6:6], xr[:], hf[:])  # xlo
                nc.vector.tensor_copy(wstat[:, 1:n6:6], cf[:])        # chi
                nc.vector.tensor_copy(hf[:], wstat[:, 1:n6:6])
                nc.vector.tensor_copy(wstat[:, 2:n6:6], hf[:])
                nc.vector.tensor_sub(cl[:], cf[:], hf[:])             # clo f32
                nc.vector.tensor_copy(wstat[:, 4:n6:6], cl[:])
                nc.vector.tensor_copy(wstat[:, 5:n6:6], cl[:])
                # rstat cols per tau: (0, -rhi, rhi, 0, -rlo, rlo); r == c
                nc.vector.tensor_scalar_mul(rstat[:, 0:n6:6], cf[:], 0.0)
                nc.vector.tensor_scalar_mul(rstat[:, 3:n6:6], cf[:], 0.0)
                nc.vector.tensor_scalar_mul(rstat[:, 1:n6:6], hf[:], -1.0)
                nc.vector.tensor_copy(rstat[:, 2:n6:6], hf[:])
                nc.vector.tensor_scalar_mul(rstat[:, 4:n6:6], cl[:], -1.0)
                nc.vector.tensor_copy(rstat[:, 5:n6:6], cl[:])

            # ================= layer 1 (row-sharded) =================
            ps1 = layer_row_sharded(stat1[:, 0:6 * T1], stat1[:, 6 * T1:12 * T1],
                                    [wh1], [wl1], T1 * MS, T1, "bank2_0", 0)
            sb1 = sp.tile([6, MS], F32, tag="sb1")
            nc.scalar.activation(sb1[:], ps1[:], ACTF.Copy)
            # local shard -> [128, 6*T2] layout (partition-major reshape DMAs)
            vec1 = sp.tile([128, 6 * T2], F32, tag="vec1")
            for v in range(6):
                nc.gpsimd.dma_start(vec1[:, v * T2:(v + 1) * T2], sb1[v:v + 1, :])
            wstat2 = sp.tile([128, 6 * T2], BF16, tag="wstat2")
            rstat2 = sp.tile([128, 6 * T2], BF16, tag="rstat2")
            boundary(vec1, T2, wstat2, rstat2, "b1")

            # ================= layer 2 (column-sharded) =================
            ps2 = [pp.tile([6, MS], F32, name=f"ps2_{n}", tag=f"bank2_{n}")
                   for n in range(NB)]
            for t_ in range(T2):
                for n in range(NB):
                    nc.tensor.matmul(ps2[n][:], wstat2[:, 6 * t_:6 * t_ + 6],
                                     slice_of(wh2, CH2, t_ * K23 + n * MS),
                                     start=(t_ == 0), stop=False)
            for t_ in range(T2):
                for n in range(NB):
                    nc.tensor.matmul(ps2[n][0:3, :], wstat2[:, 6 * t_:6 * t_ + 3],
                                     slice_of(wl2, CH2, t_ * K23 + n * MS),
                                     start=False, stop=False)
            for n in range(NB):
                nc.tensor.matmul(ps2[n][0:3, :], ones,
                                 brow[0:2, MS + n * MS:MS + (n + 1) * MS],
                                 start=False, stop=False)
            for t_ in range(T2):
                for n in range(NB):
                    a = abs_chunk(slice_of(wh2, CH2, t_ * K23 + n * MS))
                    nc.tensor.matmul(ps2[n][:], rstat2[:, 6 * t_:6 * t_ + 6],
                                     a[:], start=False, stop=(t_ == T2 - 1))
            for j in range(T3):
                src = slice_of(wh3, CH3, j * MS)
                dst = slice_of(ah3, CH3, j * MS)
                if j % 2 == 0:
                    nc.vector.scalar_tensor_tensor(dst, src, -1.0, src,
                                                   ALU.mult, ALU.max)
                else:
                    nc.scalar.activation(dst, src, ACTF.Abs)

            # ---- the one collective: AllReduce of (6, 4096) partials ----
            ar_in = dp.tile([6, K23], F32, tag="ar_in")
            ar_out = dp.tile([6, K23], F32, tag="ar_out")
            for n in range(NB):
                sb2s = sp.tile([6, MS], F32, name=f"sb2s_{n}", tag="sb2s",
                               bufs=2)
                nc.vector.tensor_copy(sb2s[:], ps2[n][:])
                nc.scalar.dma_start(ar_in[:, n * MS:(n + 1) * MS], sb2s[:])
            nc.gpsimd.collective_compute(
                "AllReduce", mybir.AluOpType.add,
                replica_groups=[list(range(N_CORES))],
                ins=[ar_in[:]], outs=[ar_out[:]],
            )
            vec2 = sp.tile([128, 6 * T3], F32, tag="vec2")
            nc.scalar.dma_start(
                vec2[:], ar_out.rearrange("v (p t) -> p v t", p=128, t=T3))
            wstat3 = sp.tile([128, 6 * T3], BF16, tag="wstat3")
            rstat3 = sp.tile([128, 6 * T3], BF16, tag="rstat3")
            boundary(vec2, T3, wstat3, rstat3, "b2")

            # ================= layer 3 (row-sharded) =================
            ps3 = layer_row_sharded(wstat3, rstat3, wh3, wl3, CH3, T3,
                                    "bank2_1", None, pre_abs=ah3)
            sb3 = sp.tile([6, MS], F32, tag="sb3")
            nc.scalar.activation(sb3[:], ps3[:], ACTF.Copy)
            nc.scalar.dma_start(out_d[:], sb3[:])
    nc.compile()
    return nc


def _bf_split(a):
    import ml_dtypes
    hi = a.astype(ml_dtypes.bfloat16)
    lo = (a - hi.astype(np.float32)).astype(ml_dtypes.bfloat16)
    return hi, lo


def _row_prep(shard, kp):
    """(rows, K) slice of W -> [128, T*rows] with wt[p, tau*rows+m] =
    W.T[p*T+tau, m], zero-padded to kp contraction rows; bf16 hi/lo pair."""
    rows, k = shard.shape
    arr = np.zeros((kp, rows), np.float32)
    arr[:k] = shard.T
    t = kp // 128
    arr = np.ascontiguousarray(arr.reshape(128, t * rows))
    return _bf_split(arr)


def _prep_inputs(x, low, high, Ws, bs):
    import ml_dtypes
    BF = ml_dtypes.bfloat16

    xn = (x - MEAN) / STD
    ln = (low - MEAN) / STD
    hn = (high - MEAN) / STD
    c0 = (ln + hn) * np.float32(0.5)
    r0 = (hn - ln) * np.float32(0.5)

    def padv(v):
        p = np.zeros(K1P, np.float32)
        p[:K1] = v
        return p.reshape(128, T1)

    xs, cs, rs = padv(xn), padv(c0), padv(r0)
    stat1 = np.zeros((128, 12 * T1), BF)
    n6 = 6 * T1
    xhi, xlo = _bf_split(xs)
    chi, clo = _bf_split(cs)
    rhi, rlo = _bf_split(rs)
    stat1[:, 0:n6:6] = xhi
    stat1[:, 1:n6:6] = chi
    stat1[:, 2:n6:6] = chi
    stat1[:, 3:n6:6] = xlo
    stat1[:, 4:n6:6] = clo
    stat1[:, 5:n6:6] = clo
    stat1[:, n6 + 1::6] = -(rhi.astype(np.float32))
    stat1[:, n6 + 2::6] = rhi
    stat1[:, n6 + 4::6] = -(rlo.astype(np.float32))
    stat1[:, n6 + 5::6] = rlo

    b2_8 = bs[1] / np.float32(8.0)

    in_maps = []
    for c in range(N_CORES):
        sl = slice(c * MS, (c + 1) * MS)
        brow = np.zeros((2, MS + K23 + 3), BF)
        b1hi, b1lo = _bf_split(bs[0][sl])
        b2hi, b2lo = _bf_split(b2_8)
        brow[0, 0:MS] = b1hi
        brow[1, 0:MS] = b1lo
        brow[0, MS:MS + K23] = b2hi
        brow[1, MS:MS + K23] = b2lo
        brow[0, MS + K23:] = np.ones(3, BF)
        brow[1, MS + K23:] = np.ones(3, BF)

        wh1, wl1 = _row_prep(Ws[0][sl], K1P)
        # L2 column shard: wt[p, tau*4096 + m] = W2[m, c*512 + p*4 + tau]
        arr2 = np.ascontiguousarray(Ws[1][:, sl].T)        # (512, 4096)
        arr2 = np.ascontiguousarray(arr2.reshape(128, T2 * K23))
        wh2, wl2 = _bf_split(arr2)
        wh3, wl3 = _row_prep(Ws[2][sl], K23)
        in_maps.append({
            "stat1": stat1, "brow": brow,
            "wh1": wh1, "wl1": wl1,
            "wh2": wh2, "wl2": wl2,
            "wh3": wh3, "wl3": wl3,
        })
    return in_maps


def _run_device(in_maps, trace=False):
    from concourse.bass_utils import run_bass_kernel_spmd

    if "nc" not in _CACHE:
        _CACHE["nc"] = _build_nc()
    return run_bass_kernel_spmd(
        _CACHE["nc"], in_maps, core_ids=list(range(N_CORES)), trace=trace,
        trace_cores=list(range(N_CORES)) if trace else None)


def kernel(x, low, high, Ws, bs, _trace=False, _res_out=None):
    x = np.asarray(x, np.float32).reshape(-1)
    low = np.asarray(low, np.float32).reshape(-1)
    high = np.asarray(high, np.float32).reshape(-1)
    Ws = [np.ascontiguousarray(np.asarray(W, np.float32)) for W in Ws]
    bs = [np.asarray(b, np.float32) for b in bs]

    in_maps = _prep_inputs(x, low, high, Ws, bs)
    res = _run_device(in_maps, trace=_trace)
    if _res_out is not None:
        _res_out.append(res)

    outs = [res.results[c]["out"] for c in range(N_CORES)]
    z3 = np.concatenate([o[0] + o[3] for o in outs])
    lo3 = np.concatenate([o[1] + o[4] for o in outs])
    hi3 = np.concatenate([o[2] + o[5] for o in outs])

    # layer-3 bias + ReLU boundary (host, fp32)
    b3 = bs[2]
    x3 = z3 + b3
    lo = lo3 + b3
    hi = hi3 + b3
    d0 = hi - lo
    slope = hi / (d0 + EPS)
    ubint = lo * hi / d0
    hi2 = slope * hi + ubint
    x3p = np.maximum(x3, np.float32(0))
    c3 = hi2 * np.float32(0.5)

    # layer 4 (10 x 4096) on host
    W4, b4 = Ws[3], bs[3]
    z4 = W4 @ x3p
    u4 = W4 @ c3
    v4 = np.abs(W4) @ c3
    x_out = (z4 + b4).reshape(1, -1).astype(np.float32)
    low_out = (u4 - v4 + b4).astype(np.float32)
    high_out = (u4 + v4 + b4).astype(np.float32)
    return (x_out, low_out, high_out)
